# revision 3
# baseline (speedup 1.0000x reference)
"""Trainium2 Bass kernel for ConstructAdjMatrix (GNN message passing).

Math (reference):
    d_x = (rowsum(adj) + 1) ** -0.5          # [N_CELL]
    d_y = (colsum(adj) + 1) ** -0.5          # [N_DRUG]
    agg_cell_lp = d_x[:,None] * adj * d_y    # [N_CELL, N_DRUG]
    agg_drug_lp = agg_cell_lp.T              # [N_DRUG, N_CELL]
    self_cell_lp = diag(1/(rowsum+1) + 1)    # [N_CELL, N_CELL]
    self_drug_lp = diag(1/(colsum+1) + 1)    # [N_DRUG, N_DRUG]

Sharding: adj row-sharded across 8 cores (1024 rows each). Row degrees are
local; column degrees need one 16KB AllReduce across the 8 cores. Each core
writes its row block of agg_cell_lp and (via PE transpose) its column block
of agg_drug_lp. The two diag outputs only need their diagonal vectors from
the device; the host assembles the (mostly zero) diag matrices.
"""

import numpy as np

from concourse import bacc, bass, masks, mybir, tile
from concourse.bass_utils import run_bass_kernel_spmd

N_CELL, N_DRUG = 8192, 4096
NC = 8                 # cores
R = N_CELL // NC       # 1024 rows per core
P = 128                # partitions
NT = R // P            # 8 row tiles of [128, 4096] per core
NB = N_DRUG // 512     # 8 psum banks for the column-sum
ND = N_DRUG // P       # 32 drug chunks of 128 for the transpose
FP32 = mybir.dt.float32
MULT = mybir.AluOpType.mult


def _build_kernel():
    nc = bacc.Bacc(
        "TRN2", target_bir_lowering=False, debug=False, num_devices=NC
    )
    adj = nc.dram_tensor("adj_block", [R, N_DRUG], FP32, kind="ExternalInput").ap()
    out1 = nc.dram_tensor("out1", [R, N_DRUG], FP32, kind="ExternalOutput").ap()
    out2 = nc.dram_tensor("out2", [N_DRUG, R], FP32, kind="ExternalOutput").ap()
    dcell = nc.dram_tensor("dcell", [R], FP32, kind="ExternalOutput").ap()
    ddrug = nc.dram_tensor("ddrug", [N_DRUG], FP32, kind="ExternalOutput").ap()

    with tile.TileContext(nc) as tc:
        _body(tc, adj, out1, out2, dcell, ddrug)
    nc.compile()
    return nc


def _body(tc, adj, out1, out2, dcell, ddrug):
    nc = tc.nc
    from contextlib import ExitStack

    with ExitStack() as ctx:
        const = ctx.enter_context(tc.tile_pool(name="const", bufs=1))
        adj_pool = ctx.enter_context(tc.tile_pool(name="adjp", bufs=1))
        sb = ctx.enter_context(tc.tile_pool(name="sb", bufs=1))
        dram = ctx.enter_context(tc.tile_pool(name="dram", bufs=1, space="DRAM"))
        cs_ctx = ExitStack()
        psum_cs = cs_ctx.enter_context(tc.tile_pool(name="psum_cs", bufs=1, space="PSUM"))

        identity = const.tile([P, P], FP32, name="identity")
        masks.make_identity(nc, identity[:])
        ones_col = const.tile([P, 1], FP32, name="ones_col")
        nc.gpsimd.memset(ones_col[:], 1.0)

        cs_in = dram.tile([N_DRUG], FP32, name="cs_in")
        cs_out = dram.tile([N_DRUG], FP32, name="cs_out", addr_space="Shared")
        dy_dram = dram.tile([N_DRUG], FP32, name="dy_dram")

        # one PSUM bank per 512-wide column stripe for the column-sum
        cs_banks = [
            psum_cs.tile([P, 512], FP32, name=f"csb{b}", tag=f"csb{b}")
            for b in range(NB)
        ]

        r_all = sb.tile([P, NT], FP32, name="r_all", tag="r_all")

        # ---- Phase A: load row tiles, rowsum, partial colsum -------------
        adj_tiles = []
        for t in range(NT):
            at = adj_pool.tile([P, N_DRUG], FP32, name=f"adj{t}", tag=f"adj{t}")
            nc.sync.dma_start(out=at[:], in_=adj[t * P : (t + 1) * P, :])
            adj_tiles.append(at)
            nc.vector.reduce_sum(
                out=r_all[:, t : t + 1], in_=at[:], axis=mybir.AxisListType.X
            )
            for b in range(NB):
                nc.tensor.matmul(
                    cs_banks[b][:1, :],
                    ones_col[:],
                    at[:, b * 512 : (b + 1) * 512],
                    start=(t == 0),
                    stop=(t == NT - 1),
                )

        # ---- row-degree vectors (local) ----------------------------------
        rp1 = sb.tile([P, NT], FP32, name="rp1", tag="rp1")
        nc.vector.tensor_scalar_add(rp1[:], r_all[:], 1.0)
        rrec = sb.tile([P, NT], FP32, name="rrec", tag="rrec")
        nc.vector.reciprocal(rrec[:], rp1[:])
        dx = sb.tile([P, NT], FP32, name="dx", tag="dx")
        nc.scalar.sqrt(dx[:], rrec[:])
        dc1 = sb.tile([P, NT], FP32, name="dc1", tag="dc1")
        nc.vector.tensor_scalar_add(dc1[:], rrec[:], 1.0)
        # dcell[t*128 + p] = dc1[p, t]
        nc.sync.dma_start(out=dcell.rearrange("(t p) -> p t", p=P), in_=dc1[:])

        # ---- colsum -> DRAM -> AllReduce ---------------------------------
        cs_row = sb.tile([1, N_DRUG], FP32, name="cs_row", tag="cs_row")
        for b in range(NB):
            nc.any.tensor_copy(cs_row[:1, b * 512 : (b + 1) * 512], cs_banks[b][:1, :])
        nc.sync.dma_start(out=cs_in[:], in_=cs_row[:1, :])
        cs_ctx.close()  # release the 8 colsum PSUM banks for the transpose pool
        nc.gpsimd.collective_compute(
            "AllReduce",
            mybir.AluOpType.add,
            replica_groups=[list(range(NC))],
            ins=[cs_in.opt()],
            outs=[cs_out.opt()],
        )

        # ---- column-degree vectors ---------------------------------------
        s_nat = sb.tile([P, N_DRUG // P], FP32, name="s_nat", tag="s_nat")
        nc.sync.dma_start(out=s_nat[:], in_=cs_out.rearrange("(p f) -> p f", p=P))
        sp1 = sb.tile([P, N_DRUG // P], FP32, name="sp1", tag="sp1")
        nc.vector.tensor_scalar_add(sp1[:], s_nat[:], 1.0)
        srec = sb.tile([P, N_DRUG // P], FP32, name="srec", tag="srec")
        nc.vector.reciprocal(srec[:], sp1[:])
        dy_nat = sb.tile([P, N_DRUG // P], FP32, name="dy_nat", tag="dy_nat")
        nc.scalar.sqrt(dy_nat[:], srec[:])
        dd1 = sb.tile([P, N_DRUG // P], FP32, name="dd1", tag="dd1")
        nc.vector.tensor_scalar_add(dd1[:], srec[:], 1.0)
        nc.sync.dma_start(out=ddrug.rearrange("(p f) -> p f", p=P), in_=dd1[:])

        # broadcast d_y across all 128 partitions: stage to DRAM, then a
        # partition-stride-0 DMA read replicates the 16KB row 128 times.
        nc.sync.dma_start(out=dy_dram.rearrange("(p f) -> p f", p=P), in_=dy_nat[:])
        dyb = const.tile([P, N_DRUG], FP32, name="dyb")
        nc.sync.dma_start(out=dyb[:], in_=dy_dram[:].partition_broadcast(P))

        # ---- Phase C: out1 = d_x * adj * d_y (in place), store -----------
        for t in range(NT):
            at = adj_tiles[t]
            nc.vector.scalar_tensor_tensor(
                out=at[:],
                in0=at[:],
                scalar=dx[:, t : t + 1],
                in1=dyb[:],
                op0=MULT,
                op1=MULT,
            )
            nc.sync.dma_start(out=out1[t * P : (t + 1) * P, :], in_=at[:])

        # ---- Phase D: out2 = out1.T via PE transpose ---------------------
        psum_tp = ctx.enter_context(tc.tile_pool(name="psum_tp", bufs=4, space="PSUM"))
        stage_pool = ctx.enter_context(tc.tile_pool(name="stage", bufs=3))
        for d in range(ND):
            stg = stage_pool.tile([P, R], FP32, name=f"stg{d}", tag="stg")
            for g in range(2):
                pt = psum_tp.tile([P, 512], FP32, name=f"pt{d}_{g}", tag="ptp")
                for t4 in range(4):
                    t = g * 4 + t4
                    nc.tensor.matmul(
                        pt[:, t4 * P : (t4 + 1) * P],
                        adj_tiles[t][:, d * P : (d + 1) * P],
                        identity[:],
                        is_transpose=True,
                    )
                nc.any.tensor_copy(stg[:, g * 512 : (g + 1) * 512], pt[:])
            nc.sync.dma_start(out=out2[d * P : (d + 1) * P, :], in_=stg[:])


_CACHE = {}


def _get_kernel():
    if "nc" not in _CACHE:
        _CACHE["nc"] = _build_kernel()
    return _CACHE["nc"]


def kernel(adj):
    adj = np.ascontiguousarray(np.asarray(adj, dtype=np.float32))
    assert adj.shape == (N_CELL, N_DRUG)
    nc = _get_kernel()
    in_maps = [{"adj_block": adj[c * R : (c + 1) * R]} for c in range(NC)]
    res = run_bass_kernel_spmd(nc, in_maps, list(range(NC))).results

    agg_cell = np.concatenate([res[c]["out1"] for c in range(NC)], axis=0)
    agg_drug = np.concatenate([res[c]["out2"] for c in range(NC)], axis=1)
    self_cell = np.zeros((N_CELL, N_CELL), np.float32)
    np.fill_diagonal(self_cell, np.concatenate([res[c]["dcell"] for c in range(NC)]))
    self_drug = np.zeros((N_DRUG, N_DRUG), np.float32)
    np.fill_diagonal(self_drug, res[0]["ddrug"])
    return (agg_cell, agg_drug, self_cell, self_drug)
